# revision 32
# baseline (speedup 1.0000x reference)
"""GQA kernel for Trainium2, sharded over 8 NeuronCores.

Problem: x[2,2048,2048] -> GQA(HQ=16 q-heads, HKV=4 kv-heads, D=128) -> out[2,2048,2048]
Sharding: core c = b*4 + h handles batch b and kv-head group h (4 q-heads).
Wq/Wk/Wv column-sharded per head group, Wo row-sharded; partial outputs
summed on host per batch.

Per-core kernel (bf16 matmul operands, fp32 PSUM accumulation), fully
software-pipelined so the PE never idles:
  1. K/V projections e-outer (all 4 psum column blocks live) so each x
     tile is consumed right after its DMA lands.
  2. V transposed via PE into va[j][key,d] tiles, with block 0's
     Q-projection matmuls interleaved.
  3. Main loop over 8 blocks (g q-head, ib 1024-query half). Per block:
     scoresT[j,i] = kT_j^T @ qT[g] (2x512 psum), exp on ACT -> et bf16,
     AV flipped orientation: va_j stationary, et moving -> rawT[d,i]
     accumulated in psum (no transposes). The NEXT block's Q-projection
     matmuls (2 per j) are interleaved into the j-loop to fill the PE
     while ACT (1.09us/tile) outpaces the per-j score+AV work (852ns).
     After AV15 the psum is copied raw to SBUF (releases the single avp
     psum buffer fast). Softmax denominator: DVE accumulates sum_j et_j,
     GPSIMD partition_all_reduce (6.7us) runs during the next block, and
     the reciprocal + normalize multiply into attnT are emitted one
     block later so nothing head-of-line blocks the DVE queue.
  4. Output projection split: nb01 pass reads only attnT[:, 0:1024]
     (ready before the last block's normalize chain completes) then
     nb23. PSUM->SBUF bf16 copies split between ACT and DVE, bf16 out.
"""

import math

import numpy as np

B = 2
N = 2048
E = 2048
HQ = 16
G = 4
HKV = 4
D = 128
FQ = G * D  # 512 q-features per group
P = 128
NB = N // 512  # 4 moving-dim chunks
ET = E // P  # 16 contraction tiles
JT = N // P  # 16 key tiles
IB2 = N // 1024  # 2 query blocks of 1024
SCALE = 1.0 / math.sqrt(D)

_CACHE: dict = {}


def _build_program():
    import concourse.bacc as bacc
    import concourse.tile as tile
    from concourse import mybir
    from concourse.bass_isa import ReduceOp
    from concourse.masks import make_identity

    f32 = mybir.dt.float32
    bf16 = mybir.dt.bfloat16
    nc = bacc.Bacc("TRN2", target_bir_lowering=False)

    xT_d = nc.dram_tensor("xT", [ET, P, N], bf16, kind="ExternalInput")
    wqT_d = nc.dram_tensor("wqT", [P, ET, FQ], bf16, kind="ExternalInput")
    wkT_d = nc.dram_tensor("wkT", [P, ET, D], bf16, kind="ExternalInput")
    wvT_d = nc.dram_tensor("wvT", [P, ET, D], bf16, kind="ExternalInput")
    woT_d = nc.dram_tensor("woT", [P, G, N], bf16, kind="ExternalInput")
    outT_d = nc.dram_tensor("outT", [ET, P, N], bf16, kind="ExternalOutput")

    BLOCKS = [(g, ib) for ib in range(IB2) for g in range(G)]

    with tile.TileContext(nc) as tc:
        with tc.tile_pool(name="persist", bufs=1) as persist, \
             tc.tile_pool(name="w1", bufs=1) as w1:
            ident = persist.tile([P, P], bf16, tag="ident")
            make_identity(nc, ident)

            qT = [persist.tile([P, N], bf16, name=f"qT{f}", tag=f"qT{f}")
                  for f in range(G)]
            kT = persist.tile([P, N], bf16, tag="kT")
            va = persist.tile([P, JT, P], bf16, tag="va")
            attnT = [persist.tile([P, N], bf16, name=f"attnT{g}", tag=f"attnT{g}")
                     for g in range(G)]
            wo_sb = persist.tile([P, G, N], bf16, tag="wo_sb")

            wq_sb = w1.tile([P, ET, FQ], bf16, tag="wq_sb")
            wk_sb = w1.tile([P, ET, D], bf16, tag="wk_sb")
            wv_sb = w1.tile([P, ET, D], bf16, tag="wv_sb")
            vTs = w1.tile([P, N], bf16, tag="vTs")
            # x tiles split 4-way across queues and issued from the idle
            # DVE/ACT sequencers (SP's per-DMA issue time would pace them);
            # weights on SP: k/v first, q next, wo last.
            xts = []
            for e in range(ET):
                if e < 4:
                    nc.sync.dma_start(out=wk_sb[:, e, :], in_=wkT_d[:, e, :])
                    nc.sync.dma_start(out=wv_sb[:, e, :], in_=wvT_d[:, e, :])
                xt = w1.tile([P, N], bf16, name=f"xt{e}", tag=f"xt{e}")
                if e == 0:
                    # finest split for the very first tile: it gates the PE
                    engs8 = [nc.sync, nc.scalar, nc.gpsimd, nc.scalar,
                             nc.sync, nc.scalar, nc.gpsimd, nc.scalar]
                    for q in range(8):
                        sl = slice(q * 256, (q + 1) * 256)
                        engs8[q].dma_start(out=xt[:, sl], in_=xT_d[e, :, sl])
                else:
                    for q in range(4):
                        sl = slice(q * 512, (q + 1) * 512)
                        eng = nc.gpsimd if q % 2 == 0 else nc.scalar
                        eng.dma_start(out=xt[:, sl], in_=xT_d[e, :, sl])
                xts.append(xt)
            for e in range(4, ET):
                nc.sync.dma_start(out=wk_sb[:, e, :], in_=wkT_d[:, e, :])
                nc.sync.dma_start(out=wv_sb[:, e, :], in_=wvT_d[:, e, :])
            for e in range(ET):
                nc.sync.dma_start(out=wq_sb[:, e, :], in_=wqT_d[:, e, :])
            for f in range(G):
                nc.sync.dma_start(out=wo_sb[:, f, :], in_=woT_d[:, f, :])

            # pull the ACT exp-table load off the critical path
            wact = persist.tile([P, 1], bf16, name="wact", tag="wact")
            nc.scalar.activation(
                wact[:], ident[:, 0:1],
                mybir.ActivationFunctionType.Exp, scale=0.001,
            )

            # ---------- phase 1: K/V projections, e-outer ----------
            with tc.tile_pool(name="pkv", bufs=1, space="PSUM") as pkv:
                kvps = [pkv.tile([P, 1024], f32, name=f"kv{nb}", tag=f"kv{nb}")
                        for nb in range(NB)]
                for e in range(ET):
                    st = e == 0
                    sp = e == ET - 1
                    for nb in range(NB):
                        sl = slice(nb * 512, (nb + 1) * 512)
                        nc.tensor.matmul(
                            kvps[nb][:, 0:512], wk_sb[:, e, :], xts[e][:, sl],
                            start=st, stop=sp,
                        )
                    for nb in range(NB):
                        sl = slice(nb * 512, (nb + 1) * 512)
                        nc.tensor.matmul(
                            kvps[nb][:, 512:1024], wv_sb[:, e, :], xts[e][:, sl],
                            start=st, stop=sp,
                        )
                # per-nb cast pairs split DVE/ACT so each psum bank pair is
                # fully read early (the next phase's psum tiles WAR against
                # these reads)
                for nb in range(NB):
                    sl = slice(nb * 512, (nb + 1) * 512)
                    nc.vector.tensor_copy(vTs[:, sl], kvps[nb][:, 512:1024])
                    nc.scalar.copy(kT[:, sl], kvps[nb][:, 0:512])

            # Q-projection emitter: 32 matmul steps per block, interleaved
            # into the previous block's j-loop (2 per j)
            with tc.tile_pool(name="qpp", bufs=2, space="PSUM") as qpp:

                def make_qsteps(g, ib):
                    steps = [(h2, e) for h2 in range(2) for e in range(ET)]
                    tiles = {}

                    def emit(k):
                        if k >= len(steps):
                            return
                        h2, e = steps[k]
                        sl = slice(ib * 1024 + h2 * 512,
                                   ib * 1024 + (h2 + 1) * 512)
                        if e == 0:
                            tiles[h2] = qpp.tile(
                                [P, 512], f32, name="qp", tag="qp"
                            )
                        nc.tensor.matmul(
                            tiles[h2][:],
                            wq_sb[:, e, g * P:(g + 1) * P], xts[e][:, sl],
                            start=(e == 0), stop=(e == ET - 1),
                        )
                        if e == ET - 1:
                            nc.vector.tensor_copy(qT[g][:, sl], tiles[h2][:])

                    return emit

                # ---- v transpose interleaved with block 0's Q-proj ----
                q0 = make_qsteps(*BLOCKS[0])
                with tc.tile_pool(name="ptr0", bufs=2, space="PSUM") as ptr0:
                    for j in range(JT):
                        tp = ptr0.tile([P, P], bf16, tag="tp0")
                        nc.tensor.transpose(
                            tp[:], vTs[:, j * P:(j + 1) * P], ident[:]
                        )
                        nc.vector.tensor_copy(va[:, j, :], tp[:])
                        q0(2 * j)
                        q0(2 * j + 1)

                # ---------- main loop ----------
                nrm_scope = tc.tile_pool(name="nrm", bufs=2)
                nrm = nrm_scope.__enter__()
                pending = None  # (g, ib, araw, den) awaiting recip+mult

                def flush_pending():
                    nonlocal pending
                    if pending is None:
                        return
                    pg, pib, praw, pden = pending
                    rec = nrm.tile([P, 1024], f32, tag="rec")
                    nc.vector.reciprocal_approx_fast(rec[:], pden[:])
                    nc.vector.tensor_mul(
                        attnT[pg][:, pib * 1024:(pib + 1) * 1024],
                        praw[:], rec[:],
                    )
                    pending = None

                with tc.tile_pool(name="et", bufs=6) as etp, \
                     tc.tile_pool(name="ps", bufs=2, space="PSUM") as ps, \
                     tc.tile_pool(name="pav", bufs=1, space="PSUM") as pav:

                    def make_p3fill():
                        # the last block has no next-block Q-projection;
                        # fill its spare PE slots with the first two output
                        # projection units (they only need ib0 attnT),
                        # reusing the idle qpp psum buffers
                        state = {}

                        def emit(k):
                            eo, m = k // 16, k % 16
                            if eo >= 2:
                                return
                            if m < 8:
                                nbh, f = m // 4, m % 4
                                if f == 0:
                                    state[(eo, nbh)] = qpp.tile(
                                        [P, 512], f32, name="qp", tag="qp"
                                    )
                                nc.tensor.matmul(
                                    state[(eo, nbh)][:],
                                    wo_sb[:, f, eo * P:(eo + 1) * P],
                                    attnT[f][:, nbh * 512:(nbh + 1) * 512],
                                    start=(f == 0), stop=(f == G - 1),
                                )
                            elif m in (8, 9):
                                nbh = m - 8
                                otf = nrm.tile([P, 512], bf16, name="otf",
                                               tag="otf")
                                nc.scalar.copy(
                                    otf[:], state[(eo, nbh)][:]
                                )
                                nc.sync.dma_start(
                                    out=outT_d[eo, :,
                                               nbh * 512:(nbh + 1) * 512],
                                    in_=otf[:],
                                )

                        return emit

                    for bi, (g, ib) in enumerate(BLOCKS):
                        i0 = ib * 1024
                        nxt = BLOCKS[bi + 1] if bi + 1 < len(BLOCKS) else None
                        qn = make_qsteps(*nxt) if nxt else make_p3fill()

                        acc = nrm.tile([P, 1024], bf16, tag="acc")
                        araw = nrm.tile([P, 1024], bf16, tag="araw")
                        avp = pav.tile([P, 1024], f32, tag="avp")
                        ets = []
                        sps_l = []

                        # previous block's normalize: deps land mid-block,
                        # nothing behind it in the DVE queue is needed sooner
                        flush_pending()

                        def scores(j):
                            sps = ps.tile([P, 1024], f32, tag="sps")
                            for half in range(2):
                                nc.tensor.matmul(
                                    sps[:, half * 512:(half + 1) * 512],
                                    kT[:, j * P:(j + 1) * P],
                                    qT[g][:, i0 + half * 512:
                                           i0 + (half + 1) * 512],
                                    start=True, stop=True,
                                )
                            sps_l.append(sps)

                        def expo(j):
                            et = etp.tile([P, 1024], bf16, tag="et")
                            nc.scalar.activation(
                                et[:], sps_l[j][:],
                                mybir.ActivationFunctionType.Exp, scale=SCALE,
                            )
                            ets.append(et)
                            if j == 0:
                                nc.vector.tensor_copy(acc[:], et[:])
                            else:
                                nc.vector.tensor_add(acc[:], acc[:], et[:])

                        def av(j):
                            for half in range(2):
                                nc.tensor.matmul(
                                    avp[:, half * 512:(half + 1) * 512],
                                    va[:, j, :],
                                    ets[j][:, half * 512:(half + 1) * 512],
                                    start=(j == 0), stop=(j == JT - 1),
                                )

                        # front-load the next block's Q-projection (3 per j)
                        # so its qT casts clear the DVE queue well before
                        # this block ends -- the next block's first scores
                        # depend on them
                        scores(0)
                        expo(0)
                        k = 0
                        for j in range(1, JT):
                            scores(j)
                            expo(j)
                            av(j - 1)
                            take = 3 if j <= 10 else 2
                            for _ in range(take):
                                if k < 32:
                                    qn(k)
                                    k += 1
                        av(JT - 1)
                        while k < 32:
                            qn(k)
                            k += 1

                        # raw copy releases the single avp psum buffer fast
                        nc.vector.tensor_copy(araw[:], avp[:])
                        den = nrm.tile([P, 1024], f32, tag="den")
                        nc.gpsimd.partition_all_reduce(
                            den[:], acc[:], P, ReduceOp.add
                        )
                        pending = (g, ib, araw, den)

            # ---------- phase 3: output projection (nb01 then nb23) ----------
            # the last block's flush is deferred past the first few eo
            # groups (they only read ib0 attnT) so the GPSIMD all-reduce
            # latency never blocks the DVE queue at the transition
            with tc.tile_pool(name="po", bufs=2, space="PSUM") as po, \
                 tc.tile_pool(name="op", bufs=3) as op:
                for half3 in range(2):
                    nbs = (0, 1) if half3 == 0 else (2, 3)
                    # eo 0,1 of the nb01 half were emitted in the last block
                    for eo in range(2 if half3 == 0 else 0, ET):
                        if half3 == 0 and eo == 5:
                            flush_pending()
                        ops_ = [po.tile([P, 512], f32, name=f"op{k}",
                                        tag=f"op{k}")
                                for k in range(2)]
                        for f in range(G):
                            for k, nb in enumerate(nbs):
                                nc.tensor.matmul(
                                    ops_[k][:],
                                    wo_sb[:, f, eo * P:(eo + 1) * P],
                                    attnT[f][:, nb * 512:(nb + 1) * 512],
                                    start=(f == 0), stop=(f == G - 1),
                                )
                        ot = op.tile([P, 1024], bf16, tag="ot")
                        for k, nb in enumerate(nbs):
                            use_scalar = (eo < 2) or ((eo + k) % 2 == 0)
                            if use_scalar:
                                nc.scalar.copy(
                                    ot[:, k * 512:(k + 1) * 512], ops_[k][:]
                                )
                            else:
                                nc.vector.tensor_copy(
                                    ot[:, k * 512:(k + 1) * 512], ops_[k][:]
                                )
                            last = half3 == 1 and eo >= ET - 3
                            if last:
                                # split the final stores across queues and
                                # issue engines so the drain tail is short
                                for q in range(4):
                                    sl = slice(nb * 512 + q * 128,
                                               nb * 512 + (q + 1) * 128)
                                    s2 = slice(k * 512 + q * 128,
                                               k * 512 + (q + 1) * 128)
                                    ie = nc.sync if q % 2 == 0 else nc.gpsimd
                                    ie.dma_start(
                                        out=outT_d[eo, :, sl], in_=ot[:, s2]
                                    )
                            else:
                                nc.sync.dma_start(
                                    out=outT_d[eo, :, nb * 512:(nb + 1) * 512],
                                    in_=ot[:, k * 512:(k + 1) * 512],
                                )
            nrm_scope.__exit__(None, None, None)
    nc.finalize()
    return nc


def _get_program():
    if "nc" not in _CACHE:
        _CACHE["nc"] = _build_program()
    return _CACHE["nc"]


def _make_in_maps(x, Wq, Wk, Wv, Wo):
    import ml_dtypes

    bf = ml_dtypes.bfloat16

    def wtile(w):  # [rows, E] -> [P, ET_rows, rows_per] tiled on partition
        r = w.shape[0]
        return np.ascontiguousarray(
            w.T.reshape(ET, P, r).transpose(1, 0, 2)
        ).astype(bf)

    xT = [
        np.ascontiguousarray(x[b].T).astype(bf).reshape(ET, P, N) for b in range(B)
    ]
    in_maps = []
    for c in range(8):
        b, h = c // HKV, c % HKV
        wo = Wo[:, h * FQ:(h + 1) * FQ].T  # [FQ, E]
        in_maps.append({
            "xT": xT[b],
            "wqT": wtile(Wq[h * FQ:(h + 1) * FQ, :]),
            "wkT": wtile(Wk[h * D:(h + 1) * D, :]),
            "wvT": wtile(Wv[h * D:(h + 1) * D, :]),
            "woT": np.ascontiguousarray(
                wo.reshape(G, P, N).transpose(1, 0, 2)
            ).astype(bf),
        })
    return in_maps


def run_spmd(in_maps, trace=False, **kw):
    from concourse.bass_utils import run_bass_kernel_spmd

    nc = _get_program()
    return run_bass_kernel_spmd(nc, in_maps, list(range(8)), trace=trace, **kw)


def kernel(x, Wq, Wk, Wv, Wo, next_token_only=0, **_ignored):
    x = np.asarray(x, dtype=np.float32)
    Wq = np.asarray(Wq, dtype=np.float32)
    Wk = np.asarray(Wk, dtype=np.float32)
    Wv = np.asarray(Wv, dtype=np.float32)
    Wo = np.asarray(Wo, dtype=np.float32)

    res = run_spmd(_make_in_maps(x, Wq, Wk, Wv, Wo))
    outs = [np.asarray(r["outT"], dtype=np.float32).reshape(E, N)
            for r in res.results]
    full = np.empty((B, N, E), np.float32)
    for b in range(B):
        acc = outs[b * HKV].copy()
        for h in range(1, HKV):
            acc += outs[b * HKV + h]
        full[b] = acc.T
    return full


# revision 33
# speedup vs baseline: 1.0067x; 1.0067x over previous
"""GQA kernel for Trainium2, sharded over 8 NeuronCores.

Problem: x[2,2048,2048] -> GQA(HQ=16 q-heads, HKV=4 kv-heads, D=128) -> out[2,2048,2048]
Sharding: core c = b*4 + h handles batch b and kv-head group h (4 q-heads).
Wq/Wk/Wv column-sharded per head group, Wo row-sharded; partial outputs
summed on host per batch.

Per-core kernel (bf16 matmul operands, fp32 PSUM accumulation), fully
software-pipelined so the PE never idles:
  1. K/V projections e-outer (all 4 psum column blocks live) so each x
     tile is consumed right after its DMA lands.
  2. V transposed via PE into va[j][key,d] tiles, with block 0's
     Q-projection matmuls interleaved.
  3. Main loop over 8 blocks (g q-head, ib 1024-query half). Per block:
     scoresT[j,i] = kT_j^T @ qT[g] (2x512 psum), exp on ACT -> et bf16,
     AV flipped orientation: va_j stationary, et moving -> rawT[d,i]
     accumulated in psum (no transposes). The NEXT block's Q-projection
     matmuls (2 per j) are interleaved into the j-loop to fill the PE
     while ACT (1.09us/tile) outpaces the per-j score+AV work (852ns).
     After AV15 the psum is copied raw to SBUF (releases the single avp
     psum buffer fast). Softmax denominator: DVE accumulates sum_j et_j,
     GPSIMD partition_all_reduce (6.7us) runs during the next block, and
     the reciprocal + normalize multiply into attnT are emitted one
     block later so nothing head-of-line blocks the DVE queue.
  4. Output projection split: nb01 pass reads only attnT[:, 0:1024]
     (ready before the last block's normalize chain completes) then
     nb23. PSUM->SBUF bf16 copies split between ACT and DVE, bf16 out.
"""

import math

import numpy as np

B = 2
N = 2048
E = 2048
HQ = 16
G = 4
HKV = 4
D = 128
FQ = G * D  # 512 q-features per group
P = 128
NB = N // 512  # 4 moving-dim chunks
ET = E // P  # 16 contraction tiles
JT = N // P  # 16 key tiles
IB2 = N // 1024  # 2 query blocks of 1024
SCALE = 1.0 / math.sqrt(D)

_CACHE: dict = {}


def _build_program():
    import concourse.bacc as bacc
    import concourse.tile as tile
    from concourse import mybir
    from concourse.bass_isa import ReduceOp
    from concourse.masks import make_identity

    f32 = mybir.dt.float32
    bf16 = mybir.dt.bfloat16
    nc = bacc.Bacc("TRN2", target_bir_lowering=False)

    xT_d = nc.dram_tensor("xT", [ET, P, N], bf16, kind="ExternalInput")
    wqT_d = nc.dram_tensor("wqT", [P, ET, FQ], bf16, kind="ExternalInput")
    wkT_d = nc.dram_tensor("wkT", [P, ET, D], bf16, kind="ExternalInput")
    wvT_d = nc.dram_tensor("wvT", [P, ET, D], bf16, kind="ExternalInput")
    woT_d = nc.dram_tensor("woT", [P, G, N], bf16, kind="ExternalInput")
    outT_d = nc.dram_tensor("outT", [ET, P, N], bf16, kind="ExternalOutput")

    BLOCKS = [(g, ib) for ib in range(IB2) for g in range(G)]

    with tile.TileContext(nc) as tc:
        with tc.tile_pool(name="persist", bufs=1) as persist, \
             tc.tile_pool(name="w1", bufs=1) as w1:
            ident = persist.tile([P, P], bf16, tag="ident")
            make_identity(nc, ident)

            qT = [persist.tile([P, N], bf16, name=f"qT{f}", tag=f"qT{f}")
                  for f in range(G)]
            kT = persist.tile([P, N], bf16, tag="kT")
            va = persist.tile([P, JT, P], bf16, tag="va")
            attnT = [persist.tile([P, N], bf16, name=f"attnT{g}", tag=f"attnT{g}")
                     for g in range(G)]
            wo_sb = persist.tile([P, G, N], bf16, tag="wo_sb")

            wq_sb = w1.tile([P, ET, FQ], bf16, tag="wq_sb")
            wk_sb = w1.tile([P, ET, D], bf16, tag="wk_sb")
            wv_sb = w1.tile([P, ET, D], bf16, tag="wv_sb")
            vTs = w1.tile([P, N], bf16, tag="vTs")
            # x tiles split 4-way across queues and issued from the idle
            # DVE/ACT sequencers (SP's per-DMA issue time would pace them);
            # weights on SP: k/v first, q next, wo last.
            xts = []
            for e in range(ET):
                if e < 4:
                    nc.sync.dma_start(out=wk_sb[:, e, :], in_=wkT_d[:, e, :])
                    nc.sync.dma_start(out=wv_sb[:, e, :], in_=wvT_d[:, e, :])
                xt = w1.tile([P, N], bf16, name=f"xt{e}", tag=f"xt{e}")
                if e == 0:
                    # finest split for the very first tile: it gates the PE
                    engs8 = [nc.sync, nc.scalar, nc.gpsimd, nc.scalar,
                             nc.sync, nc.scalar, nc.gpsimd, nc.scalar]
                    for q in range(8):
                        sl = slice(q * 256, (q + 1) * 256)
                        engs8[q].dma_start(out=xt[:, sl], in_=xT_d[e, :, sl])
                else:
                    for q in range(4):
                        sl = slice(q * 512, (q + 1) * 512)
                        eng = nc.gpsimd if q % 2 == 0 else nc.scalar
                        eng.dma_start(out=xt[:, sl], in_=xT_d[e, :, sl])
                xts.append(xt)
            for e in range(4, ET):
                nc.sync.dma_start(out=wk_sb[:, e, :], in_=wkT_d[:, e, :])
                nc.sync.dma_start(out=wv_sb[:, e, :], in_=wvT_d[:, e, :])
            for e in range(ET):
                nc.sync.dma_start(out=wq_sb[:, e, :], in_=wqT_d[:, e, :])
            for f in range(G):
                nc.sync.dma_start(out=wo_sb[:, f, :], in_=woT_d[:, f, :])

            # pull the ACT exp-table load off the critical path
            wact = persist.tile([P, 1], bf16, name="wact", tag="wact")
            nc.scalar.activation(
                wact[:], ident[:, 0:1],
                mybir.ActivationFunctionType.Exp, scale=0.001,
            )

            # ---------- phase 1: K/V projections, e-outer ----------
            with tc.tile_pool(name="pkv", bufs=1, space="PSUM") as pkv:
                kvps = [pkv.tile([P, 1024], f32, name=f"kv{nb}", tag=f"kv{nb}")
                        for nb in range(NB)]
                for e in range(ET):
                    st = e == 0
                    sp = e == ET - 1
                    for nb in range(NB):
                        sl = slice(nb * 512, (nb + 1) * 512)
                        nc.tensor.matmul(
                            kvps[nb][:, 0:512], wk_sb[:, e, :], xts[e][:, sl],
                            start=st, stop=sp,
                        )
                    for nb in range(NB):
                        sl = slice(nb * 512, (nb + 1) * 512)
                        nc.tensor.matmul(
                            kvps[nb][:, 512:1024], wv_sb[:, e, :], xts[e][:, sl],
                            start=st, stop=sp,
                        )
                # per-nb cast pairs split DVE/ACT so each psum bank pair is
                # fully read early (the next phase's psum tiles WAR against
                # these reads)
                for nb in range(NB):
                    sl = slice(nb * 512, (nb + 1) * 512)
                    nc.vector.tensor_copy(vTs[:, sl], kvps[nb][:, 512:1024])
                    nc.scalar.copy(kT[:, sl], kvps[nb][:, 0:512])

            # Q-projection emitter: 32 matmul steps per block, interleaved
            # into the previous block's j-loop (2 per j)
            with tc.tile_pool(name="qpp", bufs=2, space="PSUM") as qpp:

                def make_qsteps(g, ib):
                    steps = [(h2, e) for h2 in range(2) for e in range(ET)]
                    tiles = {}

                    def emit(k):
                        if k >= len(steps):
                            return
                        h2, e = steps[k]
                        sl = slice(ib * 1024 + h2 * 512,
                                   ib * 1024 + (h2 + 1) * 512)
                        if e == 0:
                            tiles[h2] = qpp.tile(
                                [P, 512], f32, name="qp", tag="qp"
                            )
                        nc.tensor.matmul(
                            tiles[h2][:],
                            wq_sb[:, e, g * P:(g + 1) * P], xts[e][:, sl],
                            start=(e == 0), stop=(e == ET - 1),
                        )
                        if e == ET - 1:
                            nc.vector.tensor_copy(qT[g][:, sl], tiles[h2][:])

                    return emit

                # ---- v transpose interleaved with block 0's Q-proj ----
                q0 = make_qsteps(*BLOCKS[0])
                with tc.tile_pool(name="ptr0", bufs=2, space="PSUM") as ptr0:
                    for j in range(JT):
                        tp = ptr0.tile([P, P], bf16, tag="tp0")
                        nc.tensor.transpose(
                            tp[:], vTs[:, j * P:(j + 1) * P], ident[:]
                        )
                        nc.vector.tensor_copy(va[:, j, :], tp[:])
                        q0(2 * j)
                        q0(2 * j + 1)

                # ---------- main loop ----------
                nrm_scope = tc.tile_pool(name="nrm", bufs=2)
                nrm = nrm_scope.__enter__()
                pending = None  # (g, ib, araw, den) awaiting recip+mult

                def flush_pending():
                    nonlocal pending
                    if pending is None:
                        return
                    pg, pib, praw, pden = pending
                    rec = nrm.tile([P, 1024], f32, tag="rec")
                    nc.vector.reciprocal_approx_fast(rec[:], pden[:])
                    nc.vector.tensor_mul(
                        attnT[pg][:, pib * 1024:(pib + 1) * 1024],
                        praw[:], rec[:],
                    )
                    pending = None

                with tc.tile_pool(name="et", bufs=6) as etp, \
                     tc.tile_pool(name="ps", bufs=2, space="PSUM") as ps, \
                     tc.tile_pool(name="pav", bufs=1, space="PSUM") as pav:

                    def make_p3fill():
                        # the last block has no next-block Q-projection;
                        # fill its spare PE slots with the first two output
                        # projection units (they only need ib0 attnT),
                        # reusing the idle qpp psum buffers
                        state = {}

                        def emit(k):
                            eo, m = k // 16, k % 16
                            if eo >= 2:
                                return
                            if m < 8:
                                nbh, f = m // 4, m % 4
                                if f == 0:
                                    state[(eo, nbh)] = qpp.tile(
                                        [P, 512], f32, name="qp", tag="qp"
                                    )
                                nc.tensor.matmul(
                                    state[(eo, nbh)][:],
                                    wo_sb[:, f, eo * P:(eo + 1) * P],
                                    attnT[f][:, nbh * 512:(nbh + 1) * 512],
                                    start=(f == 0), stop=(f == G - 1),
                                )
                            elif m in (8, 9):
                                nbh = m - 8
                                otf = nrm.tile([P, 512], bf16, name="otf",
                                               tag="otf")
                                nc.scalar.copy(
                                    otf[:], state[(eo, nbh)][:]
                                )
                                nc.sync.dma_start(
                                    out=outT_d[eo, :,
                                               nbh * 512:(nbh + 1) * 512],
                                    in_=otf[:],
                                )

                        return emit

                    for bi, (g, ib) in enumerate(BLOCKS):
                        i0 = ib * 1024
                        nxt = BLOCKS[bi + 1] if bi + 1 < len(BLOCKS) else None
                        qn = make_qsteps(*nxt) if nxt else make_p3fill()

                        acc = nrm.tile([P, 1024], bf16, tag="acc")
                        araw = nrm.tile([P, 1024], bf16, tag="araw")
                        avp = pav.tile([P, 1024], f32, tag="avp")
                        ets = []
                        sps_l = []

                        # previous block's normalize: deps land mid-block,
                        # nothing behind it in the DVE queue is needed sooner
                        flush_pending()

                        def scores(j):
                            sps = ps.tile([P, 1024], f32, tag="sps")
                            for half in range(2):
                                nc.tensor.matmul(
                                    sps[:, half * 512:(half + 1) * 512],
                                    kT[:, j * P:(j + 1) * P],
                                    qT[g][:, i0 + half * 512:
                                           i0 + (half + 1) * 512],
                                    start=True, stop=True,
                                )
                            sps_l.append(sps)

                        def expo(j):
                            et = etp.tile([P, 1024], bf16, tag="et")
                            nc.scalar.activation(
                                et[:], sps_l[j][:],
                                mybir.ActivationFunctionType.Exp, scale=SCALE,
                            )
                            ets.append(et)
                            if j == 0:
                                nc.vector.tensor_copy(acc[:], et[:])
                            else:
                                nc.vector.tensor_add(acc[:], acc[:], et[:])

                        def av(j):
                            for half in range(2):
                                nc.tensor.matmul(
                                    avp[:, half * 512:(half + 1) * 512],
                                    va[:, j, :],
                                    ets[j][:, half * 512:(half + 1) * 512],
                                    start=(j == 0), stop=(j == JT - 1),
                                )

                        # front-load the next block's Q-projection (3 per j)
                        # so its qT casts clear the DVE queue well before
                        # this block ends -- the next block's first scores
                        # depend on them
                        scores(0)
                        expo(0)
                        k = 0
                        for j in range(1, JT):
                            scores(j)
                            expo(j)
                            av(j - 1)
                            take = 3 if j <= 10 else 2
                            for _ in range(take):
                                if k < 32:
                                    qn(k)
                                    k += 1
                        av(JT - 1)
                        while k < 32:
                            qn(k)
                            k += 1

                        # raw copy releases the single avp psum buffer fast
                        nc.vector.tensor_copy(araw[:], avp[:])
                        den = nrm.tile([P, 1024], f32, tag="den")
                        nc.gpsimd.partition_all_reduce(
                            den[:], acc[:], P, ReduceOp.add
                        )
                        pending = (g, ib, araw, den)

                    # ------ phase 3: output projection (nb01 then nb23) ------
                    # emitted inside the main pool scope, reusing the sps/et
                    # pool tags: opening fresh pools here inserts a drain
                    # against the GPSIMD all-reduce. The last block's flush
                    # is deferred past the first few eo groups (they only
                    # read ib0 attnT).
                    for half3 in range(2):
                        nbs = (0, 1) if half3 == 0 else (2, 3)
                        # eo 0,1 of the nb01 half ran inside the last block
                        for eo in range(2 if half3 == 0 else 0, ET):
                            if half3 == 0 and eo == 5:
                                flush_pending()
                            ops_ = ps.tile([P, 1024], f32, name="sps",
                                           tag="sps")
                            for f in range(G):
                                for k, nb in enumerate(nbs):
                                    nc.tensor.matmul(
                                        ops_[:, k * 512:(k + 1) * 512],
                                        wo_sb[:, f, eo * P:(eo + 1) * P],
                                        attnT[f][:, nb * 512:(nb + 1) * 512],
                                        start=(f == 0), stop=(f == G - 1),
                                    )
                            ot = etp.tile([P, 1024], bf16, name="et", tag="et")
                            for k, nb in enumerate(nbs):
                                use_scalar = (eo + k) % 2 == 0
                                if use_scalar:
                                    nc.scalar.copy(
                                        ot[:, k * 512:(k + 1) * 512],
                                        ops_[:, k * 512:(k + 1) * 512],
                                    )
                                else:
                                    nc.vector.tensor_copy(
                                        ot[:, k * 512:(k + 1) * 512],
                                        ops_[:, k * 512:(k + 1) * 512],
                                    )
                                last = half3 == 1 and eo >= ET - 3
                                if last:
                                    # split the final stores across queues
                                    # and issue engines for a short drain
                                    for q in range(4):
                                        sl = slice(nb * 512 + q * 128,
                                                   nb * 512 + (q + 1) * 128)
                                        s2 = slice(k * 512 + q * 128,
                                                   k * 512 + (q + 1) * 128)
                                        ie = (nc.sync if q % 2 == 0
                                              else nc.gpsimd)
                                        ie.dma_start(
                                            out=outT_d[eo, :, sl],
                                            in_=ot[:, s2],
                                        )
                                else:
                                    nc.sync.dma_start(
                                        out=outT_d[eo, :,
                                                   nb * 512:(nb + 1) * 512],
                                        in_=ot[:, k * 512:(k + 1) * 512],
                                    )
            nrm_scope.__exit__(None, None, None)
    nc.finalize()
    return nc


def _get_program():
    if "nc" not in _CACHE:
        _CACHE["nc"] = _build_program()
    return _CACHE["nc"]


def _make_in_maps(x, Wq, Wk, Wv, Wo):
    import ml_dtypes

    bf = ml_dtypes.bfloat16

    def wtile(w):  # [rows, E] -> [P, ET_rows, rows_per] tiled on partition
        r = w.shape[0]
        return np.ascontiguousarray(
            w.T.reshape(ET, P, r).transpose(1, 0, 2)
        ).astype(bf)

    xT = [
        np.ascontiguousarray(x[b].T).astype(bf).reshape(ET, P, N) for b in range(B)
    ]
    in_maps = []
    for c in range(8):
        b, h = c // HKV, c % HKV
        wo = Wo[:, h * FQ:(h + 1) * FQ].T  # [FQ, E]
        in_maps.append({
            "xT": xT[b],
            "wqT": wtile(Wq[h * FQ:(h + 1) * FQ, :]),
            "wkT": wtile(Wk[h * D:(h + 1) * D, :]),
            "wvT": wtile(Wv[h * D:(h + 1) * D, :]),
            "woT": np.ascontiguousarray(
                wo.reshape(G, P, N).transpose(1, 0, 2)
            ).astype(bf),
        })
    return in_maps


def run_spmd(in_maps, trace=False, **kw):
    from concourse.bass_utils import run_bass_kernel_spmd

    nc = _get_program()
    return run_bass_kernel_spmd(nc, in_maps, list(range(8)), trace=trace, **kw)


def kernel(x, Wq, Wk, Wv, Wo, next_token_only=0, **_ignored):
    x = np.asarray(x, dtype=np.float32)
    Wq = np.asarray(Wq, dtype=np.float32)
    Wk = np.asarray(Wk, dtype=np.float32)
    Wv = np.asarray(Wv, dtype=np.float32)
    Wo = np.asarray(Wo, dtype=np.float32)

    res = run_spmd(_make_in_maps(x, Wq, Wk, Wv, Wo))
    outs = [np.asarray(r["outT"], dtype=np.float32).reshape(E, N)
            for r in res.results]
    full = np.empty((B, N, E), np.float32)
    for b in range(B):
        acc = outs[b * HKV].copy()
        for h in range(1, HKV):
            acc += outs[b * HKV + h]
        full[b] = acc.T
    return full
